# revision 10
# baseline (speedup 1.0000x reference)
"""Trainium2 Bass kernel for nn_MemNet (memory-network attention block).

Computation (per row r of B*R=5120 rows):
    fused  = tanh(cat(img, ques) @ W_fuse.T + b_fuse)          [5120, 512]
    s_j    = sum_d hist[r,j,d] * fused[r,d] * w_att[d] + b_att [5120, 10]
    attn   = softmax(s, axis=1)
    he     = sum_j attn[r,j] * hist[r,j,:]                     [5120, 512]
    he     = tanh(he @ W_hist.T + b_hist)
    out    = fused + he   -> reshape [512, 10, 512]

Strategy: pure data parallel over the leading 5120 rows -> 640 rows/core on
8 cores, 5 row-tiles of 128 rows each.  Weights replicated.  Activations for
the big matmul are pre-transposed on the host so the contraction dim lands on
SBUF partitions (no on-chip transposes for matmul 1).  The attention middle
stays in natural [row, feature] layout on the Vector engine with fused
scalar_tensor_tensor ops.  hist_embed is transposed on the PE to feed
matmul 2.  Biases are folded into PSUM via a ones-row matmul (b_att is
dropped entirely: softmax is shift-invariant).  Both matmuls run in fp32r
(full-rate fp32).  The per-row-tile work is software-pipelined in two
stages so PE (matmul1 of tile t+1) overlaps the DVE attention of tile t.
"""

import os

import numpy as np

# ---- problem constants (hardcoded per contract) ----
B = 512
R = 10
BR = B * R  # 5120
IMG = 2048
D = 512
FUSION = IMG + D  # 2560
NCORES = 8
ROWS = BR // NCORES  # 640
NRT = ROWS // 128  # 5 row tiles / core
KC = FUSION // 128  # 20 contraction chunks for matmul1
DC = D // 128  # 4 contraction chunks for matmul2

# packed-constants column offsets (floats per partition)
OFF_WATT = 0
OFF_EYE = OFF_WATT + D  # 512
OFF_BFUSE = OFF_EYE + 128  # 640
OFF_BHIST = OFF_BFUSE + D  # 1152
OFF_ONES = OFF_BHIST + D  # 1664
OFF_EYE16 = OFF_ONES + 128  # 1792 (64 f32 cols = 128 bf16 cols)
CCOLS = OFF_EYE16 + 64  # 1856

# matmul dtype: "bf16" (full-rate PE, halves weight/activation DMA),
# "fp32r" (full-rate fp32, ~1e-4 relative error) or "fp32" (4x slower PE).
MM_DTYPE = os.environ.get("MEMNET_MM_DTYPE", "bf16")
# hist dtype: "bf16" halves the dominant hist DMA and doubles DVE score
# throughput (packed 2x mode); "fp32" is bit-conservative.
HIST_DTYPE = os.environ.get("MEMNET_HIST_DTYPE", "bf16")
# residual add on the (idle) GpSimd engine instead of the busy Vector engine
POOL_ADD = bool(int(os.environ.get("MEMNET_POOL_ADD", "1")))

_PROGRAM = None
LAST_RESULTS = None  # BassKernelResults of the most recent run (for profiling)


def _build_program():
    import concourse.bacc as bacc
    import concourse.mybir as mybir
    import concourse.tile as tile

    dt = mybir.dt
    f32 = dt.float32
    Alu = mybir.AluOpType
    Act = mybir.ActivationFunctionType
    Ax = mybir.AxisListType

    # matmul operands live at mmdt end-to-end (DRAM + SBUF); for fp32r the
    # BIR verifier requires fp32r-rounded producers, which DMA/ACT satisfy.
    mmdt = {"bf16": dt.bfloat16, "fp32r": dt.float32r, "fp32": f32}[MM_DTYPE]
    hdt = dt.bfloat16 if HIST_DTYPE == "bf16" else f32

    nc = bacc.Bacc("TRN2", target_bir_lowering=False, debug=False)

    # per-core inputs.  w1 carries W_fuse^T (20 chunks) + W_hist^T (4 chunks)
    # in one DMA; all small f32 constants travel in one packed DMA.  Fewer
    # DMA-completion semaphores -> fewer waits on consumers.
    fvt = nc.dram_tensor("fvt", [NRT, 128, KC, 128], mmdt, kind="ExternalInput")
    hist = nc.dram_tensor("hist", [ROWS, R, D], hdt, kind="ExternalInput")
    w1 = nc.dram_tensor("w1", [128, KC + DC, D], mmdt, kind="ExternalInput")
    cpack = nc.dram_tensor("cpack", [128, CCOLS], f32, kind="ExternalInput")
    out = nc.dram_tensor("out", [ROWS, D], f32, kind="ExternalOutput")

    with tile.TileContext(nc) as tc:
        with (
            tc.tile_pool(name="const", bufs=1) as cpool,
            tc.tile_pool(name="act", bufs=2) as apool,
            tc.tile_pool(name="histp", bufs=3) as hpool,
            tc.tile_pool(name="fusedp", bufs=3) as fpool,
            tc.tile_pool(name="work", bufs=2) as wpool,
            tc.tile_pool(name="outp", bufs=2) as opool,
            tc.tile_pool(name="small", bufs=2) as spool,
            tc.tile_pool(name="ps1", bufs=2, space="PSUM") as pp1,
            tc.tile_pool(name="pst", bufs=2, space="PSUM") as ppt,
            tc.tile_pool(name="ps2", bufs=2, space="PSUM") as pp2,
        ):
            # consts + weights stream on the ACT HWDGE ring (in pieces, so
            # matmul1 can start after the first piece); activations use the
            # sync ring.  The two rings drain in parallel.
            cp_sb = cpool.tile([128, CCOLS], f32)
            nc.scalar.dma_start(cp_sb[:], cpack[:])
            WPC = 6  # weight chunks per DMA piece; pieces alternate rings
            w1p = []
            for n, i in enumerate(range(0, KC + DC, WPC)):
                t = cpool.tile([128, WPC, D], mmdt, tag=f"w1p{i}")
                eng = nc.scalar if n % 2 == 0 else nc.sync
                eng.dma_start(t[:], w1[:, i : i + WPC, :])
                w1p.append(t)

            def w1_ap(c):
                return w1p[c // WPC][:, c % WPC, :]

            watt_ap = cp_sb[:, OFF_WATT : OFF_WATT + D]
            eye_ap = cp_sb[:, OFF_EYE : OFF_EYE + 128]
            eye16_ap = cp_sb[:, OFF_EYE16 : OFF_EYE16 + 64].bitcast(dt.bfloat16)
            bfuse_ap = cp_sb[0:1, OFF_BFUSE : OFF_BFUSE + D]
            bhist_ap = cp_sb[0:1, OFF_BHIST : OFF_BHIST + D]
            ones_ap = cp_sb[0:1, OFF_ONES : OFF_ONES + 128]

            h_tiles = {}
            fused_tiles = {}

            def stage_a(rt):
                """loads + matmul1 + tanh -> fused[rt]"""
                a_sb = apool.tile([128, KC, 128], mmdt, tag="a")
                nc.sync.dma_start(a_sb[:], fvt[rt])
                h_sb = hpool.tile([128, R, D], hdt, tag="h")
                nc.sync.dma_start(h_sb[:], hist[rt * 128 : (rt + 1) * 128])
                h_tiles[rt] = h_sb

                # bias matmul leads the accumulation group: it carries the
                # PSUM WAW wait, so the k=0 matmul only waits on its DMA.
                ps1 = pp1.tile([128, D], f32, tag="ps1")
                nc.tensor.matmul(ps1[:], ones_ap, bfuse_ap, start=True, stop=False)
                for k in range(KC):
                    nc.tensor.matmul(
                        ps1[:],
                        a_sb[:, k, :],
                        w1_ap(k),
                        start=False,
                        stop=(k == KC - 1),
                    )
                fused_sb = fpool.tile([128, D], f32, tag="fused")
                nc.scalar.activation(fused_sb[:], ps1[:], Act.Tanh)
                fused_tiles[rt] = fused_sb

            def stage_b(rt):
                """attention + matmul2 + residual + store for row-tile rt"""
                h_sb = h_tiles.pop(rt)
                fused_sb = fused_tiles.pop(rt)

                wfused_sb = wpool.tile([128, D], hdt, tag="wfused")
                nc.vector.tensor_mul(wfused_sb[:], fused_sb[:], watt_ap)

                # scores_j = sum_d hist_j*wfused (b_att dropped: softmax is
                # shift-invariant so it cannot affect the output)
                scores = spool.tile([128, R], f32, tag="scores")
                scratch = wpool.tile([128, D], hdt, tag="scratch")
                for j in range(R):
                    nc.vector.scalar_tensor_tensor(
                        out=scratch[:],
                        in0=h_sb[:, j, :],
                        scalar=0.0,
                        in1=wfused_sb[:],
                        op0=Alu.bypass,
                        op1=Alu.mult,
                        accum_out=scores[:, j : j + 1],
                    )

                # softmax over the R=10 scores
                negmax = spool.tile([128, 1], f32, tag="negmax")
                nc.vector.reduce_max(negmax[:], scores[:], axis=Ax.X, negate=True)
                probs = spool.tile([128, R], f32, tag="probs")
                sumexp = spool.tile([128, 1], f32, tag="sumexp")
                nc.scalar.activation(
                    probs[:],
                    scores[:],
                    Act.Exp,
                    bias=negmax[:],
                    scale=1.0,
                    accum_out=sumexp[:],
                )
                rcp = spool.tile([128, 1], f32, tag="rcp")
                nc.vector.reciprocal(rcp[:], sumexp[:])
                attn = spool.tile([128, R], f32, tag="attn")
                nc.vector.tensor_scalar_mul(attn[:], probs[:], rcp[:])

                # weighted sum of hist rows.
                if HIST_DTYPE == "bf16":
                    # ACT produces tmp_j = attn_j * h_j; DVE tree-adds them
                    # with fast bf16 tensor_tensor ops.
                    tmp = []
                    for j in range(R):
                        t = wpool.tile([128, D], hdt, tag=f"tmp{j}")
                        nc.scalar.activation(
                            t[:], h_sb[:, j, :], Act.Copy, scale=attn[:, j : j + 1]
                        )
                        tmp.append(t)
                    stride = 1
                    while stride < R:
                        for j in range(0, R - stride, 2 * stride):
                            nc.vector.tensor_add(
                                tmp[j][:], tmp[j][:], tmp[j + stride][:]
                            )
                        stride *= 2
                    cur = tmp[0]
                    tdt, teye = hdt, eye16_ap
                else:
                    acc_a = wpool.tile([128, D], f32, tag="acca")
                    acc_b = wpool.tile([128, D], f32, tag="accb")
                    nc.scalar.activation(
                        acc_a[:], h_sb[:, 0, :], Act.Copy, scale=attn[:, 0:1]
                    )
                    cur, nxt = acc_a, acc_b
                    for j in range(1, R):
                        nc.vector.scalar_tensor_tensor(
                            out=nxt[:],
                            in0=h_sb[:, j, :],
                            scalar=attn[:, j : j + 1],
                            in1=cur[:],
                            op0=Alu.mult,
                            op1=Alu.add,
                        )
                        cur, nxt = nxt, cur
                    tdt, teye = f32, eye_ap

                # transpose hist_embed on PE; PSUM->SBUF eviction on ACT
                het_sb = wpool.tile([128, DC, 128], mmdt, tag="het")
                for c in range(DC):
                    pst = ppt.tile([128, 128], tdt, tag="pst")
                    nc.tensor.transpose(pst[:], cur[:, c * 128 : (c + 1) * 128], teye)
                    nc.scalar.activation(het_sb[:, c, :], pst[:], Act.Copy)

                # matmul2: he = tanh(he @ W_hist.T + b_hist)
                ps2 = pp2.tile([128, D], f32, tag="ps2")
                nc.tensor.matmul(ps2[:], ones_ap, bhist_ap, start=True, stop=False)
                for c in range(DC):
                    nc.tensor.matmul(
                        ps2[:],
                        het_sb[:, c, :],
                        w1_ap(KC + c),
                        start=False,
                        stop=(c == DC - 1),
                    )
                he_sb = wpool.tile([128, D], f32, tag="he")
                nc.scalar.activation(he_sb[:], ps2[:], Act.Tanh)

                # residual add + store (on GpSimd: DVE is the bottleneck)
                out_sb = opool.tile([128, D], f32, tag="out")
                if POOL_ADD:
                    nc.gpsimd.tensor_add(out_sb[:], fused_sb[:], he_sb[:])
                else:
                    nc.vector.tensor_add(out_sb[:], fused_sb[:], he_sb[:])
                nc.scalar.dma_start(out[rt * 128 : (rt + 1) * 128, :], out_sb[:])

            # two-stage software pipeline: PE work of tile rt overlaps the
            # DVE attention chain of tile rt-1.
            for rt in range(NRT):
                stage_a(rt)
                if rt >= 1:
                    stage_b(rt - 1)
            stage_b(NRT - 1)

    nc.compile()
    return nc


def get_program():
    global _PROGRAM
    if _PROGRAM is None:
        _PROGRAM = _build_program()
    return _PROGRAM


def shard_inputs(img, ques, hist, W_fuse, b_fuse, w_att, b_att, W_hist, b_hist):
    """Host-side layout preprocessing + sharding.  Returns list of in_maps."""
    f = np.float32
    img = np.asarray(img, f)
    ques = np.asarray(ques, f)
    hist = np.asarray(hist, f)
    W_fuse = np.asarray(W_fuse, f)
    W_hist = np.asarray(W_hist, f)

    import ml_dtypes

    mm_np = ml_dtypes.bfloat16 if MM_DTYPE == "bf16" else f
    h_np = ml_dtypes.bfloat16 if HIST_DTYPE == "bf16" else f

    fv = np.concatenate([img, ques], axis=1)  # [5120, 2560]
    # fvt[core][rt, p, c, r] = fv[core*640 + rt*128 + r, c*128 + p]
    fvt = np.ascontiguousarray(
        fv.reshape(NCORES, NRT, 128, KC, 128).transpose(0, 1, 4, 3, 2).astype(mm_np)
    )
    hist_sh = np.ascontiguousarray(hist.reshape(NCORES, ROWS, R, D).astype(h_np))

    # w1[p, c, n] = W_fuse[n, c*128 + p] for c < KC, then W_hist chunks
    w1a = W_fuse.T.reshape(KC, 128, D).transpose(1, 0, 2)
    w1b = W_hist.T.reshape(DC, 128, D).transpose(1, 0, 2)
    w1 = np.ascontiguousarray(np.concatenate([w1a, w1b], axis=1).astype(mm_np))

    cpack = np.zeros((128, CCOLS), f)
    cpack[:, OFF_WATT : OFF_WATT + D] = np.asarray(w_att, f)[None, :]
    cpack[:, OFF_EYE : OFF_EYE + 128] = np.eye(128, dtype=f)
    cpack[:, OFF_BFUSE : OFF_BFUSE + D] = np.asarray(b_fuse, f)[None, :]
    cpack[:, OFF_BHIST : OFF_BHIST + D] = np.asarray(b_hist, f)[None, :]
    cpack[:, OFF_ONES : OFF_ONES + 128] = 1.0
    eye16 = np.eye(128, dtype=ml_dtypes.bfloat16)
    cpack[:, OFF_EYE16 : OFF_EYE16 + 64] = (
        eye16.view(np.uint16).reshape(128, 64, 2).view(np.uint32).reshape(128, 64)
    ).view(np.float32)

    return [
        {
            "fvt": fvt[c],
            "hist": hist_sh[c],
            "w1": w1,
            "cpack": cpack,
        }
        for c in range(NCORES)
    ]


def kernel(
    img,
    ques,
    hist,
    W_fuse,
    b_fuse,
    w_att,
    b_att,
    W_hist,
    b_hist,
    batch_size=B,
    num_rounds=R,
    **_unused,
):
    global LAST_RESULTS
    from concourse.bass_utils import run_bass_kernel_spmd

    nc = get_program()
    in_maps = shard_inputs(
        img, ques, hist, W_fuse, b_fuse, w_att, b_att, W_hist, b_hist
    )
    trace = bool(int(os.environ.get("MEMNET_TRACE", "0")))
    res = run_bass_kernel_spmd(
        nc, in_maps, core_ids=list(range(NCORES)), trace=trace
    )
    LAST_RESULTS = res
    full = np.concatenate([res.results[c]["out"] for c in range(NCORES)], axis=0)
    return full.reshape(B, R, D).astype(np.float32)



# revision 13
# speedup vs baseline: 1.2605x; 1.2605x over previous
"""Trainium2 Bass kernel for nn_MemNet (memory-network attention block).

Computation (per row r of B*R=5120 rows):
    fused  = tanh(cat(img, ques) @ W_fuse.T + b_fuse)          [5120, 512]
    s_j    = sum_d hist[r,j,d] * fused[r,d] * w_att[d] + b_att [5120, 10]
    attn   = softmax(s, axis=1)
    he     = sum_j attn[r,j] * hist[r,j,:]                     [5120, 512]
    he     = tanh(he @ W_hist.T + b_hist)
    out    = fused + he   -> reshape [512, 10, 512]

Strategy: pure data parallel over the leading 5120 rows -> 640 rows/core on
8 cores, 5 row-tiles of 128 rows each.  Weights replicated.  All inputs are
prefetched with one early burst of large DMAs (everything fits in SBUF), so
the DMA rings drain back-to-back at full HBM bandwidth for the whole kernel.

Engine plan per row-tile:
  - matmul1 on PE: stationary = activation chunks (pre-transposed on host),
    moving = W_fuse^T chunks; bias via a K=1 ones-row matmul placed LAST in
    the accumulation group (so the group's first matmul carries any PSUM
    WAW wait, and the K=1 matmul is never a cold-start leader).
  - scores on DVE via fused tensor_tensor_reduce (mul + free-axis reduce in
    one op per round j).
  - softmax: DVE reduce_max / ACT exp(accum) / DVE reciprocal.
  - weighted sum on PE: he = sum_j diag(attn_j) @ hist_j.  The diagonal
    matrices are built with one DVE tensor_scalar op each
    (eye_bf16 * probs_j * rcp), which also folds in the softmax division.
  - he -> transpose on PE -> matmul2 -> tanh -> residual add -> store.

This moves the attention-weighted sum from ACT/DVE (where it serialized the
whole kernel) onto the PE, which hides under the DMA roofline.
"""

import os

import numpy as np

# ---- problem constants (hardcoded per contract) ----
B = 512
R = 10
BR = B * R  # 5120
IMG = 2048
D = 512
FUSION = IMG + D  # 2560
NCORES = 8
ROWS = BR // NCORES  # 640
NRT = ROWS // 128  # 5 row tiles / core
KC = FUSION // 128  # 20 contraction chunks for matmul1
DC = D // 128  # 4 contraction chunks for matmul2
WPC = 6  # w1 chunks per DMA piece
NWP = (KC + DC + WPC - 1) // WPC  # 4 pieces

# packed-constants column offsets (f32 columns; bf16 data is bitcast-packed)
OFF_WATT = 0  # watt bf16 replicated [128, 512] -> 256 f32 cols
OFF_EYE16 = OFF_WATT + 256  # eye bf16 [128, 128] -> 64 f32 cols
OFF_BFUSE = OFF_EYE16 + 64  # b_fuse bf16 [1, 512] -> 256 cols (row 0 only)
OFF_BHIST = OFF_BFUSE + 256
OFF_ONES = OFF_BHIST + 256  # ones bf16 [1, 128] -> 64 cols (row 0 only)
CCOLS = OFF_ONES + 64  # 896

_PROGRAM = None
LAST_RESULTS = None  # BassKernelResults of the most recent run (for profiling)


def _build_program():
    import concourse.bacc as bacc
    import concourse.mybir as mybir
    import concourse.tile as tile

    dt = mybir.dt
    f32 = dt.float32
    bf16 = dt.bfloat16
    Alu = mybir.AluOpType
    Act = mybir.ActivationFunctionType
    Ax = mybir.AxisListType

    nc = bacc.Bacc("TRN2", target_bir_lowering=False, debug=False)

    fvt = nc.dram_tensor("fvt", [NRT, 128, KC, 128], bf16, kind="ExternalInput")
    hist = nc.dram_tensor("hist", [ROWS, R, D], bf16, kind="ExternalInput")
    w1 = nc.dram_tensor("w1", [128, KC + DC, D], bf16, kind="ExternalInput")
    cpack = nc.dram_tensor("cpack", [128, CCOLS], f32, kind="ExternalInput")
    out = nc.dram_tensor("out", [ROWS, D], f32, kind="ExternalOutput")

    with tile.TileContext(nc) as tc:
        with (
            tc.tile_pool(name="const", bufs=1) as cpool,
            tc.tile_pool(name="act", bufs=1) as apool,
            tc.tile_pool(name="histp", bufs=1) as hpool,
            tc.tile_pool(name="fusedp", bufs=3) as fpool,
            tc.tile_pool(name="work", bufs=2) as wpool,
            tc.tile_pool(name="outp", bufs=2) as opool,
            tc.tile_pool(name="small", bufs=2) as spool,
            tc.tile_pool(name="ps1", bufs=2, space="PSUM") as pp1,
            tc.tile_pool(name="psA", bufs=2, space="PSUM") as ppA,
            tc.tile_pool(name="pst", bufs=2, space="PSUM") as ppt,
            tc.tile_pool(name="ps2", bufs=2, space="PSUM") as pp2,
        ):
            # ---- prefetch: queue every load upfront, in consumption order,
            # on the sync HWDGE ring so the SDMA engines drain back-to-back.
            cp_sb = cpool.tile([128, CCOLS], f32)
            nc.sync.dma_start(cp_sb[:], cpack[:])

            a_tiles = []
            h_tiles = []
            w1p = []

            def load_fvt(rt):
                t = apool.tile([128, KC, 128], bf16, tag=f"a{rt}")
                nc.sync.dma_start(t[:], fvt[rt])
                a_tiles.append(t)

            def load_hist(rt):
                t = hpool.tile([128, R, D], bf16, tag=f"h{rt}")
                h0 = R // 2
                nc.sync.dma_start(
                    t[:, 0:h0, :], hist[rt * 128 : (rt + 1) * 128, 0:h0, :]
                )
                nc.sync.dma_start(
                    t[:, h0:R, :], hist[rt * 128 : (rt + 1) * 128, h0:R, :]
                )
                h_tiles.append(t)

            load_fvt(0)
            for i in range(NWP):
                lo = i * WPC
                hi = min(lo + WPC, KC + DC)
                t = cpool.tile([128, hi - lo, D], bf16, tag=f"w1p{i}")
                nc.sync.dma_start(t[:], w1[:, lo:hi, :])
                w1p.append(t)
            load_fvt(1)
            load_hist(0)
            for rt in range(2, NRT):
                load_fvt(rt)
                load_hist(rt - 1)
            load_hist(NRT - 1)

            def w1_ap(c):
                return w1p[c // WPC][:, c % WPC, :]

            watt_ap = cp_sb[:, OFF_WATT : OFF_WATT + 256].bitcast(bf16)
            eye16_ap = cp_sb[:, OFF_EYE16 : OFF_EYE16 + 64].bitcast(bf16)
            bfuse_ap = cp_sb[0:1, OFF_BFUSE : OFF_BFUSE + 256].bitcast(bf16)
            bhist_ap = cp_sb[0:1, OFF_BHIST : OFF_BHIST + 256].bitcast(bf16)
            ones_ap = cp_sb[0:1, OFF_ONES : OFF_ONES + 64].bitcast(bf16)

            fused_tiles = {}
            attn_tiles = {}  # rt -> (probs, rcp) folded into diag tiles
            diag_tiles = {}

            def stage_a(rt):
                """matmul1 + tanh -> fused[rt] (f32)"""
                a_sb = a_tiles[rt]
                ps1 = pp1.tile([128, D], f32, tag="ps1")
                for k in range(KC):
                    nc.tensor.matmul(
                        ps1[:], a_sb[:, k, :], w1_ap(k), start=(k == 0), stop=False
                    )
                # bias last: K=1 ones-row x b_fuse row
                nc.tensor.matmul(ps1[:], ones_ap, bfuse_ap, start=False, stop=True)
                fused_sb = fpool.tile([128, D], f32, tag="fused")
                nc.scalar.activation(fused_sb[:], ps1[:], Act.Tanh)
                fused_tiles[rt] = fused_sb

            def stage_b(rt):
                """scores + softmax + diag build for row-tile rt"""
                h_sb = h_tiles[rt]
                fused_sb = fused_tiles[rt]

                wfused_sb = wpool.tile([128, D], bf16, tag="wfused")
                nc.vector.tensor_mul(wfused_sb[:], fused_sb[:], watt_ap)

                # scores_j = sum_d hist_j * wfused  (b_att dropped: softmax is
                # shift-invariant so it cannot affect the output)
                scores = spool.tile([128, R], f32, tag="scores")
                scratch = wpool.tile([128, D], bf16, tag="scratch")
                use_ttr = bool(int(os.environ.get("MEMNET_TTR", "0")))
                for j in range(R):
                    if use_ttr:
                        nc.vector.tensor_tensor_reduce(
                            out=scratch[:],
                            in0=h_sb[:, j, :],
                            in1=wfused_sb[:],
                            scale=1.0,
                            scalar=0.0,
                            op0=Alu.mult,
                            op1=Alu.add,
                            accum_out=scores[:, j : j + 1],
                        )
                    else:
                        nc.vector.scalar_tensor_tensor(
                            out=scratch[:],
                            in0=h_sb[:, j, :],
                            scalar=0.0,
                            in1=wfused_sb[:],
                            op0=Alu.bypass,
                            op1=Alu.mult,
                            accum_out=scores[:, j : j + 1],
                        )

                # softmax over the R=10 scores; the 1/sumexp is folded into
                # the diagonal build below.
                negmax = spool.tile([128, 1], f32, tag="negmax")
                nc.vector.reduce_max(negmax[:], scores[:], axis=Ax.X, negate=True)
                probs = spool.tile([128, R], f32, tag="probs")
                sumexp = spool.tile([128, 1], f32, tag="sumexp")
                nc.scalar.activation(
                    probs[:],
                    scores[:],
                    Act.Exp,
                    bias=negmax[:],
                    scale=1.0,
                    accum_out=sumexp[:],
                )
                rcp = spool.tile([128, 1], f32, tag="rcp")
                nc.vector.reciprocal(rcp[:], sumexp[:])

                # diag_j = eye * attn_j  (bf16, one DVE op per round)
                diag = wpool.tile([128, R, 128], bf16, tag="diag")
                if bool(int(os.environ.get("MEMNET_TS2", "0"))):
                    for j in range(R):
                        nc.vector.tensor_scalar(
                            out=diag[:, j, :],
                            in0=eye16_ap,
                            scalar1=probs[:, j : j + 1],
                            scalar2=rcp[:],
                            op0=Alu.mult,
                            op1=Alu.mult,
                        )
                else:
                    attn = spool.tile([128, R], f32, tag="attn")
                    nc.vector.tensor_scalar_mul(attn[:], probs[:], rcp[:])
                    for j in range(R):
                        nc.vector.tensor_scalar_mul(
                            diag[:, j, :], eye16_ap, attn[:, j : j + 1]
                        )
                diag_tiles[rt] = diag

            def stage_c(rt):
                """weighted sum (PE diag matmuls) + matmul2 + residual + store"""
                h_sb = h_tiles[rt]
                diag = diag_tiles.pop(rt)
                fused_sb = fused_tiles.pop(rt)

                # he = sum_j diag(attn_j) @ hist_j   [128, 512] in PSUM
                psA = ppA.tile([128, D], f32, tag="psA")
                for j in range(R):
                    nc.tensor.matmul(
                        psA[:],
                        diag[:, j, :],
                        h_sb[:, j, :],
                        start=(j == 0),
                        stop=(j == R - 1),
                    )
                he_sb = wpool.tile([128, D], bf16, tag="he")
                nc.scalar.activation(he_sb[:], psA[:], Act.Copy)

                # transpose he on PE (4 chunks into one full PSUM bank — the
                # f32 container pads the tile to 2 KiB so no other PSUM tile
                # can share the bank), evict once
                pst_f = ppt.tile([128, DC, 128], f32, tag="pst")
                pst = pst_f.bitcast(bf16)  # [128, DC, 256]
                for c in range(DC):
                    nc.tensor.transpose(
                        pst[:, c, 0:128], he_sb[:, c * 128 : (c + 1) * 128], eye16_ap
                    )
                het_sb = wpool.tile([128, DC, 128], bf16, tag="het")
                nc.vector.tensor_copy(het_sb[:], pst[:, :, 0:128])

                # matmul2: he2 = tanh(he @ W_hist.T + b_hist)
                ps2 = pp2.tile([128, D], f32, tag="ps2")
                for c in range(DC):
                    nc.tensor.matmul(
                        ps2[:], het_sb[:, c, :], w1_ap(KC + c), start=(c == 0),
                        stop=False,
                    )
                nc.tensor.matmul(ps2[:], ones_ap, bhist_ap, start=False, stop=True)
                t2_sb = wpool.tile([128, D], f32, tag="t2")
                nc.scalar.activation(t2_sb[:], ps2[:], Act.Tanh)

                # residual add + store
                out_sb = opool.tile([128, D], f32, tag="out")
                nc.vector.tensor_add(out_sb[:], fused_sb[:], t2_sb[:])
                nc.scalar.dma_start(out[rt * 128 : (rt + 1) * 128, :], out_sb[:])

            # 3-stage software pipeline across row tiles
            stage_a(0)
            stage_a(1)
            stage_b(0)
            for rt in range(2, NRT):
                stage_a(rt)
                stage_b(rt - 1)
                stage_c(rt - 2)
            stage_b(NRT - 1)
            stage_c(NRT - 2)
            stage_c(NRT - 1)

    nc.compile()
    return nc


def get_program():
    global _PROGRAM
    if _PROGRAM is None:
        _PROGRAM = _build_program()
    return _PROGRAM


def _bf16_pack(arr_bf16):
    """View a bf16 array with even last dim as packed f32 (for cpack)."""
    u16 = arr_bf16.view(np.uint16)
    return u16.reshape(*u16.shape[:-1], u16.shape[-1] // 2, 2).view(np.uint32)[
        ..., 0
    ].view(np.float32)


def shard_inputs(img, ques, hist, W_fuse, b_fuse, w_att, b_att, W_hist, b_hist):
    """Host-side layout preprocessing + sharding.  Returns list of in_maps."""
    import ml_dtypes

    f = np.float32
    bf = ml_dtypes.bfloat16
    img = np.asarray(img, f)
    ques = np.asarray(ques, f)
    hist = np.asarray(hist, f)
    W_fuse = np.asarray(W_fuse, f)
    W_hist = np.asarray(W_hist, f)

    fv = np.concatenate([img, ques], axis=1)  # [5120, 2560]
    # fvt[core][rt, p, c, r] = fv[core*640 + rt*128 + r, c*128 + p]
    fvt = np.ascontiguousarray(
        fv.reshape(NCORES, NRT, 128, KC, 128).transpose(0, 1, 4, 3, 2).astype(bf)
    )
    hist_sh = np.ascontiguousarray(hist.reshape(NCORES, ROWS, R, D).astype(bf))

    # w1[p, c, n] = W_fuse[n, c*128 + p] for c < KC, then W_hist chunks
    w1a = W_fuse.T.reshape(KC, 128, D).transpose(1, 0, 2)
    w1b = W_hist.T.reshape(DC, 128, D).transpose(1, 0, 2)
    w1 = np.ascontiguousarray(np.concatenate([w1a, w1b], axis=1).astype(bf))

    cpack = np.zeros((128, CCOLS), f)
    watt_rep = np.broadcast_to(np.asarray(w_att, f).astype(bf)[None, :], (128, D))
    cpack[:, OFF_WATT : OFF_WATT + 256] = _bf16_pack(np.ascontiguousarray(watt_rep))
    eye16 = np.eye(128, dtype=bf)
    cpack[:, OFF_EYE16 : OFF_EYE16 + 64] = _bf16_pack(eye16)
    cpack[0, OFF_BFUSE : OFF_BFUSE + 256] = _bf16_pack(
        np.asarray(b_fuse, f).astype(bf)[None, :]
    )[0]
    cpack[0, OFF_BHIST : OFF_BHIST + 256] = _bf16_pack(
        np.asarray(b_hist, f).astype(bf)[None, :]
    )[0]
    cpack[0, OFF_ONES : OFF_ONES + 64] = _bf16_pack(np.ones((1, 128), bf))[0]

    return [
        {
            "fvt": fvt[c],
            "hist": hist_sh[c],
            "w1": w1,
            "cpack": cpack,
        }
        for c in range(NCORES)
    ]


def kernel(
    img,
    ques,
    hist,
    W_fuse,
    b_fuse,
    w_att,
    b_att,
    W_hist,
    b_hist,
    batch_size=B,
    num_rounds=R,
    **_unused,
):
    global LAST_RESULTS
    from concourse.bass_utils import run_bass_kernel_spmd

    nc = get_program()
    in_maps = shard_inputs(
        img, ques, hist, W_fuse, b_fuse, w_att, b_att, W_hist, b_hist
    )
    trace = bool(int(os.environ.get("MEMNET_TRACE", "0")))
    res = run_bass_kernel_spmd(
        nc, in_maps, core_ids=list(range(NCORES)), trace=trace
    )
    LAST_RESULTS = res
    full = np.concatenate([res.results[c]["out"] for c in range(NCORES)], axis=0)
    return full.reshape(B, R, D).astype(np.float32)
